# revision 1
# baseline (speedup 1.0000x reference)
"""DistanceFromAnswerLoss on 8 Trainium2 NeuronCores.

out = 0.1 * sum_{b,c} mask[b,c] * exp(input[b,c])
  mask[b,c] = |c - t_b| / sqrt(sum_c (c - t_b)^2),  mask = 0 where t_b == 0

Transposed data-parallel layout: rows are sorted by t on the host and
sharded 512/core; each core's shard is sent TRANSPOSED (columns on
partitions): x_T[p, slot, b] = x[b, c] with c = perm[slot]*128 + p.

Because each core's 512 sorted targets span a narrow band, a contiguous
16-block window (2048 columns) covers every t.  Column blocks outside the
window have sign(c - t_b) constant over the whole core, so their entire
contribution collapses onto the TensorE:
  sum_{c in agg} |c-t_b| e[c,b] = A~[b] + (m - t_b) * S~[b]
with A~ = sum +-(c-m) e and S~ = sum +-e accumulated by 48 two-column
matmuls into one [2, 512] PSUM region (stationary cols [+-(c-m), +-1]
from the host).  Only the 16 window blocks need elementwise |c-t|*e on
the DVE, reduced by ones-column matmuls.  The result is shipped as three
dot products (host adds them): <scale, P_window>, <scale, A~>,
<scale*(m-t), S~>.

Layout/scheduling notes (from perfetto traces):
 - aggregate tiles stream FIRST, the two window tiles LAST, so the PE
   backlog (agg matmuls cost ~6.0us/tile vs ~5.3-5.9us DMA per tile)
   drains during the cheap window tiles and the tail stays short;
 - all 16.8 MB of x goes on the sync HWDGE ring in consumption order
   (the idle Sync engine absorbs buffer-waits); constants ride the
   scalar ring so they land immediately; ACT never issues DMAs;
 - DVE lanes cannot cross partitions: the S~ row is moved to partition
   0 via a tiny SBUF->SBUF DMA after a PSUM->SBUF copy;
 - the elementwise reduction runs on the DVE (stt + accum_out, scale
   pre-folded into the weights) instead of PE ones-matmuls; the full
   window tile sits at stream position 2 so its 4.4us stt overlaps DMA;
 - measured: DMA stream ~43-52us (335-399 GB/s), ACT ~36us (14x exp),
   DVE ~32us, PE ~21us; HW exec ~63-65us (baseline was 75-80us; the
   device speed drifts run-to-run, compare traces not wall numbers).
"""

import sys
from contextlib import ExitStack

import numpy as np

sys.path.insert(0, "/opt/trn_rl_repo")

import concourse.bass as bass
import concourse.tile as tile
from concourse import bacc, mybir
from concourse.bass_utils import run_bass_kernel_spmd

B = 4096
C = 8192
N_CORES = 8
ROWS = B // N_CORES          # 512 rows (free dim) per core
NQ = C // 128                # 64 column blocks of 128 (partition dim)
NS = 16                      # elementwise window blocks (contiguous in c)
NAGG = NQ - NS               # 48 aggregate blocks
W = 4096                     # x tile width in columns-of-x_T layout
NT = (NQ * ROWS) // W        # 8 tiles of [128, 4096] per core
SLOTS_PER_TILE = W // ROWS   # 8 q-slots per tile
NSTRIP = 4                   # strips for first/last tile
COEFF = 0.1

F32 = mybir.dt.float32
BF16 = mybir.dt.bfloat16
Af = mybir.ActivationFunctionType
Op = mybir.AluOpType


def _build() -> bass.Bass:
    nc = bacc.Bacc("TRN2", target_bir_lowering=False, debug=False)
    x = nc.declare_dram_parameter("x", [128, NQ * ROWS], F32, isOutput=False)
    tb = nc.declare_dram_parameter("tb", [128, ROWS], F32, isOutput=False)
    cvals = nc.declare_dram_parameter("cvals", [128, NS], F32, isOutput=False)
    wv = nc.declare_dram_parameter("wv", [128, 2 * NAGG], F32, isOutput=False)
    scale = nc.declare_dram_parameter("scale", [1, ROWS], F32, isOutput=False)
    sc2 = nc.declare_dram_parameter("sc2", [1, ROWS], F32, isOutput=False)
    out = nc.declare_dram_parameter("out", [1, 5], F32, isOutput=True)

    SW = W // NSTRIP             # strip width (1024 = 2 slots)
    SPS = SW // ROWS             # slots per strip (2)

    with tile.TileContext(nc) as tc, ExitStack() as ctx:
        const_pool = ctx.enter_context(tc.tile_pool(name="const", bufs=1))
        xpool = ctx.enter_context(tc.tile_pool(name="x", bufs=4))
        epool = ctx.enter_context(tc.tile_pool(name="e", bufs=3))
        ppool = ctx.enter_context(tc.tile_pool(name="p", bufs=2))
        spool = ctx.enter_context(tc.tile_pool(name="s", bufs=1))
        psum_pool = ctx.enter_context(tc.tile_pool(name="ps", bufs=1, space="PSUM"))

        # --- small constant inputs on the scalar HWDGE ring, so they land
        # immediately instead of queueing behind 16 MiB of x on sync ------
        tbt = const_pool.tile([128, ROWS], F32)
        nc.scalar.dma_start(out=tbt[:], in_=tb[:, :])
        cvt = const_pool.tile([128, NS], F32)
        nc.scalar.dma_start(out=cvt[:], in_=cvals[:, :])
        wvt = const_pool.tile([128, 2 * NAGG], F32)
        nc.scalar.dma_start(out=wvt[:], in_=wv[:, :])
        sct = const_pool.tile([1, ROWS], F32)
        nc.scalar.dma_start(out=sct[:], in_=scale[:, :])
        sc2t = const_pool.tile([1, ROWS], F32)
        nc.scalar.dma_start(out=sc2t[:], in_=sc2[:, :])

        # --- all x-tile DMAs on the sync HWDGE ring, in consumption order;
        # the idle Sync engine absorbs every buffer-wait + issue so the ACT
        # engine never touches DMA mid-stream.  tile 0 (elem slots 0..7) as
        # strips, tiles 1..6 (agg) full, tile 7 (elem slots 8..15) strips
        x0 = [xpool.tile([128, SW], F32, tag="xs0", name=f"x0_{s}") for s in range(NSTRIP)]
        for s in range(NSTRIP):
            nc.sync.dma_start(out=x0[s][:], in_=x[:, s * SW:(s + 1) * SW])
        xmid = []
        for j in range(1, NT - 1):
            xt = xpool.tile([128, W], F32, tag="xm", bufs=3)
            nc.sync.dma_start(out=xt[:], in_=x[:, j * W:(j + 1) * W])
            xmid.append(xt)
        X7W = [SW, SW, SW, SW // 2, SW // 2]   # tapered final strips
        x7 = []
        c0 = (NT - 1) * W
        for s, w7 in enumerate(X7W):
            xt7 = xpool.tile([128, w7], F32, tag=f"xs7_{w7}", name=f"x7_{s}")
            nc.sync.dma_start(out=xt7[:], in_=x[:, c0:c0 + w7])
            x7.append(xt7)
            c0 += w7

        # device-side casts + derived constants
        tbf = const_pool.tile([128, ROWS], BF16)
        nc.vector.tensor_copy(tbf[:], tbt[:])
        wvb = const_pool.tile([128, 2 * NAGG], BF16)
        nc.vector.tensor_copy(wvb[:], wvt[:])
        onec = const_pool.tile([128, 1], BF16)
        nc.vector.memset(onec[:], 1.0)
        # scale broadcast [128, ROWS] built on-device: ones[1,128]^T @ sct
        bc_ones = const_pool.tile([1, 128], F32)
        nc.vector.memset(bc_ones[:], 1.0)
        bcps = psum_pool.tile([128, ROWS], F32, tag="bc")
        nc.tensor.matmul(bcps[:], bc_ones[:], sct[:], start=True, stop=True)
        scbb = const_pool.tile([128, ROWS], BF16)
        nc.vector.tensor_copy(scbb[:], bcps[:])

        # elementwise weights W = |t - c| * scale_b for the 16 window
        # slots, built as max(d, -d) with d = (t - c) * scale folded into
        # one stt per slot (scale >= 0; abs is not a DVE ISA op)
        wt = const_pool.tile([128, NS * ROWS], BF16)
        HS = NS // 2

        def build_w_half(h):
            dh = const_pool.tile([128, HS * ROWS], BF16, tag="dh", name=f"dh{h}")
            for s in range(HS):
                nc.vector.scalar_tensor_tensor(
                    dh[:, s * ROWS:(s + 1) * ROWS], tbf[:],
                    cvt[:, h * HS + s:h * HS + s + 1], scbb[:],
                    op0=Op.subtract, op1=Op.mult,
                )
            nh = const_pool.tile([128, HS * ROWS], BF16, tag="nh", name=f"nh{h}")
            nc.vector.tensor_scalar(nh[:], dh[:], -1.0, None, op0=Op.mult)
            nc.vector.tensor_tensor(
                wt[:, h * HS * ROWS:(h + 1) * HS * ROWS], dh[:], nh[:], op=Op.max
            )

        build_w_half(0)   # window slots 0..7 (tile 2, needed mid-stream)

        # PSUM: one [2, ROWS] region accumulates A~ (partition 0) and S~
        # (partition 1) via paired stationary columns, plus two regions for
        # the elementwise row sums of tile 0 and tile 7
        asps = psum_pool.tile([2, ROWS], F32, tag="pas")
        psa2 = psum_pool.tile([1, ROWS], F32, tag="psa2")
        pss2 = psum_pool.tile([1, ROWS], F32, tag="pss2")
        pe11 = psum_pool.tile([1, 1], F32, tag="pe11")

        def agg_mm(et_ap, k, q, last):
            nc.tensor.matmul(
                asps[:], wvb[:, 2 * q:2 * q + 2],
                et_ap[:, k * ROWS:(k + 1) * ROWS],
                start=(q == 0), stop=last,
            )

        NAGG1 = NAGG - SLOTS_PER_TILE          # region-1 aggregate blocks

        def elem_tile(xt, half, col):
            et = epool.tile([128, W], BF16, tag="em", bufs=4, name=f"ete{half}")
            nc.scalar.activation(et[:], xt[:], Af.Exp)
            pt = ppool.tile([128, W], BF16, tag="pp6", name=f"pte{half}")
            nc.vector.scalar_tensor_tensor(
                pt[:], wt[:, half * W:(half + 1) * W], 0.0, et[:],
                op0=Op.add, op1=Op.mult, accum_out=pacc[:, col:col + 1],
            )

        pacc = spool.tile([128, 2], F32)

        # --- tile 0: aggregate q 0..7 (strips, fast pipeline fill) --------
        for s in range(NSTRIP):
            es = epool.tile([128, SW], BF16, tag="es0", name=f"es0_{s}")
            nc.scalar.activation(es[:], x0[s][:], Af.Exp)
            for k in range(SPS):
                agg_mm(es, k, s * SPS + k, False)

        # --- tile 1: aggregate q 8..15 ------------------------------------
        et1 = epool.tile([128, W], BF16, tag="em", bufs=4, name="et1")
        nc.scalar.activation(et1[:], xmid[0][:], Af.Exp)
        for k in range(SLOTS_PER_TILE):
            agg_mm(et1, k, SLOTS_PER_TILE + k, False)

        # --- tile 2: elementwise window slots 0..7 (mid-stream) -----------
        elem_tile(xmid[1], 0, 0)
        build_w_half(1)   # window slots 8..15 (tile 5)

        # --- tiles 3,4,5: aggregate q 16..39 (ends region 1 early) --------
        for j in (3, 4, 5):
            et = epool.tile([128, W], BF16, tag="em", bufs=4)
            nc.scalar.activation(et[:], xmid[j - 1][:], Af.Exp)
            for k in range(SLOTS_PER_TILE):
                agg_mm(et, k, (j - 1) * SLOTS_PER_TILE + k,
                       j == 5 and k == SLOTS_PER_TILE - 1)

        # region-1 combine, fully mid-stream: copy PSUM->SBUF aligned, move
        # the S~ row to partition 0 by SBUF->SBUF DMA, then two dot products
        res = spool.tile([1, 5], F32)
        assb = spool.tile([2, ROWS], F32)
        nc.vector.tensor_copy(assb[:], asps[:])
        srow = spool.tile([1, ROWS], F32)
        nc.sync.dma_start(out=srow[:], in_=assb[1:2, :])
        j2 = spool.tile([1, ROWS], F32)
        nc.vector.scalar_tensor_tensor(
            j2[:], assb[0:1, :], 0.0, sct[:], op0=Op.add, op1=Op.mult,
            accum_out=res[:, 1:2],
        )
        j3 = spool.tile([1, ROWS], F32)
        nc.vector.scalar_tensor_tensor(
            j3[:], srow[:], 0.0, sc2t[:], op0=Op.add, op1=Op.mult,
            accum_out=res[:, 2:3],
        )

        # --- tile 6: elementwise window slots 8..15 -----------------------
        elem_tile(xmid[5], 1, 1)

        # elementwise total: cross-partition sum, before the last strips
        paccs = spool.tile([128, 1], F32)
        nc.vector.tensor_reduce(
            paccs[:], pacc[:], axis=mybir.AxisListType.X, op=Op.add
        )
        paccb = spool.tile([128, 1], BF16)
        nc.vector.tensor_copy(paccb[:], paccs[:])
        nc.tensor.matmul(pe11[:], onec[:], paccb[:], start=True, stop=True)
        nc.vector.tensor_copy(res[:, 0:1], pe11[:])

        # --- tile 7: aggregate q 40..47 (tapered strips) into separate
        # 1-col PSUM regions: no cross-partition move in the tail ----------
        q = NAGG1
        for s, w7 in enumerate(X7W):
            es = epool.tile([128, w7], BF16, tag=f"es7_{w7}", name=f"es7_{s}")
            nc.scalar.activation(es[:], x7[s][:], Af.Exp)
            for k in range(w7 // ROWS):
                rhs = es[:, k * ROWS:(k + 1) * ROWS]
                nc.tensor.matmul(
                    psa2[:], wvb[:, 2 * q:2 * q + 1], rhs,
                    start=(q == NAGG1), stop=(q == NAGG - 1),
                )
                nc.tensor.matmul(
                    pss2[:], wvb[:, 2 * q + 1:2 * q + 2], rhs,
                    start=(q == NAGG1), stop=(q == NAGG - 1),
                )
                q += 1
        j4 = spool.tile([1, ROWS], F32)
        nc.vector.scalar_tensor_tensor(
            j4[:], psa2[:], 0.0, sct[:], op0=Op.add, op1=Op.mult,
            accum_out=res[:, 3:4],
        )
        j5 = spool.tile([1, ROWS], F32)
        nc.vector.scalar_tensor_tensor(
            j5[:], pss2[:], 0.0, sc2t[:], op0=Op.add, op1=Op.mult,
            accum_out=res[:, 4:5],
        )

        # --- tail: store the five partial dot products --------------------
        nc.sync.dma_start(out=out[:, :], in_=res[:])

    nc.finalize()
    return nc


_NC = None


def _get_nc() -> bass.Bass:
    global _NC
    if _NC is None:
        _NC = _build()
    return _NC


def _plan(target: np.ndarray):
    """Sort rows by target; per core pick a contiguous 16-block window
    covering all its targets and a block permutation [window | rest]."""
    t = np.asarray(target).astype(np.int64).reshape(B)
    order = np.argsort(t, kind="stable")
    plans = []
    for k in range(N_CORES):
        rows = order[k * ROWS:(k + 1) * ROWS]
        tc = t[rows]
        blo, bhi = int(tc.min()) >> 7, int(tc.max()) >> 7
        span = bhi - blo + 1
        assert span <= NS, f"target spread too wide for window: {span} blocks"
        # center the window on the target band (keeps |c - m| minimal)
        wlo = min(max(blo - (NS - span) // 2, 0), NQ - NS)
        assert wlo <= blo and bhi < wlo + NS
        win = np.arange(wlo, wlo + NS)
        rest = np.array([q for q in range(NQ) if q < wlo or q >= wlo + NS])
        plans.append((rows, tc, win, rest))
    return plans


def make_in_maps(input: np.ndarray, target: np.ndarray) -> list[dict]:
    xf = np.asarray(input, dtype=np.float32)
    plans = _plan(target)
    # row norm (exact closed form): sum_c (c-t)^2 = C*t^2 - 2*t*S1 + S2
    s1 = (C - 1) * C // 2
    s2 = (C - 1) * C * (2 * C - 1) // 6
    in_maps = []
    p128 = np.arange(128, dtype=np.float64)
    for rows, tc, win, rest in plans:
        # agg tiles 0,1; window 0..7 at tile 2; agg 3,4,5; window 8..15
        # at tile 6; tile 7 = region-2 agg strips
        perm = np.concatenate([rest[:16], win[:8], rest[16:40], win[8:], rest[40:]])
        # x_T[p, slot, b] = x[b, perm[slot]*128 + p]
        xk = xf[rows].reshape(ROWS, NQ, 128)[:, perm, :]
        xT = np.ascontiguousarray(xk.transpose(2, 1, 0)).reshape(128, NQ * ROWS)
        tb = np.ascontiguousarray(
            np.broadcast_to(tc.astype(np.float32), (128, ROWS))
        )
        cvals = (win[None, :] * 128 + p128[:, None]).astype(np.float32)
        m = float(win[0] * 128 + (NS * 128) / 2.0)
        # aggregate stationary columns: [ +-(c - m), +-1 ] per block
        sgn = np.where(rest * 128 >= win[-1] * 128 + 128, 1.0, -1.0)
        cagg = rest[None, :] * 128 + p128[:, None] - m      # [128, NAGG]
        wvc = np.empty((128, 2 * NAGG), dtype=np.float32)
        wvc[:, 0::2] = cagg * sgn[None, :]
        wvc[:, 1::2] = np.broadcast_to(sgn[None, :], (128, NAGG))
        norm = np.sqrt(
            C * tc.astype(np.float64) ** 2 - 2.0 * tc * s1 + s2
        )
        sc64 = COEFF / np.maximum(norm, 1e-12) * (tc != 0)
        sc = sc64.astype(np.float32)
        sc2v = (sc64 * (m - tc.astype(np.float64))).astype(np.float32)
        in_maps.append({
            "x": xT,
            "tb": tb,
            "cvals": np.ascontiguousarray(cvals),
            "wv": wvc,
            "scale": np.ascontiguousarray(sc.reshape(1, ROWS)),
            "sc2": np.ascontiguousarray(sc2v.reshape(1, ROWS)),
        })
    return in_maps


def run(input: np.ndarray, target: np.ndarray, trace: bool = False, tmpdir=None):
    nc = _get_nc()
    in_maps = make_in_maps(input, target)
    res = run_bass_kernel_spmd(
        nc, in_maps, list(range(N_CORES)), trace=trace, tmpdir=tmpdir
    )
    total = np.float32(0.0)
    for r in res.results:
        total += np.float32(r["out"].reshape(-1).sum())
    return np.asarray(total, dtype=np.float32), res


def kernel(input: np.ndarray, target: np.ndarray) -> np.ndarray:
    out, _ = run(input, target)
    return out



# revision 7
# speedup vs baseline: 1.6711x; 1.6711x over previous
"""DistanceFromAnswerLoss on 8 Trainium2 NeuronCores — v2.

out = 0.1 * sum_{b,c} mask[b,c] * exp(input[b,c])
  mask[b,c] = |c - t_b| / sqrt(sum_c (c - t_b)^2),  mask = 0 where t_b == 0

Data-parallel: rows sorted by t on the host, 512/core, shipped transposed
(columns on partitions) in bf16.  Per core a contiguous 16-block window
(2048 columns) covers every t; outside it sign(c - t_b) is constant per
block, so with m = window center:

  sum_{c in agg} |c-t_b| e[c,b] = A~[b] + (m - t_b) * S~[b]
    A~ = sum +-(c-m) e,  S~ = sum +-e     (per-block 2-col matmuls)

For the 16 window blocks the weight is folded into exp's argument on the
host (log-trick):  |c-t| e^x = e^{x + ln|c-t|}, so the window payload is
bf16(x + ln|c-t|) and its contribution W~[b] = sum_win e rides the A~ PSUM
row via a [1, 0] stationary column (both rows are finally dotted with the
same per-row scales sct / sc2 = scale, scale*(m-t)).

exp runs entirely on the DVE as a Schraudolph bf16-bitcast:
  e^x ~= bitcast_bf16(int16(floor(x * 128/ln2 + (127*128 - CADJ))))
one 4x-mode tensor_scalar per tile (~1.1us/[128,4096]), leaving ACT idle
and the kernel DMA-bound (~8.4 MB bf16 at ~390 GB/s).  CADJ calibrated
so the sawtooth's geometric mean is 1 (floor semantics).

Device per core: 8 tile DMAs -> 8 DVE tensor_scalar -> 64 matmuls into
one [2,512] PSUM group -> 2 DVE dot products -> out [1,2].
"""

import sys
from contextlib import ExitStack

import numpy as np
import ml_dtypes

sys.path.insert(0, "/opt/trn_rl_repo")

import concourse.bass as bass
import concourse.tile as tile
from concourse import bacc, mybir
from concourse.bass_utils import run_bass_kernel_spmd

B = 4096
C = 8192
N_CORES = 8
ROWS = B // N_CORES          # 512 rows (free dim) per core
NQ = C // 128                # 64 column blocks of 128 (partition dim)
NS = 16                      # window blocks (log-baked weights, contiguous)
NAGG = NQ - NS               # 48 aggregate blocks
W = 4096                     # x tile width (8 slots of ROWS)
NT = (NQ * ROWS) // W        # 8 tiles of [128, 4096] per core
SLOTS = W // ROWS            # 8 slots per tile
COEFF = 0.1

SCHR_SCALE = float(np.float32(128.0 / np.log(2.0)))
CADJ = 6.83                  # floor-calibrated sawtooth centering
SCHR_BIAS = float(np.float32(127.0 * 128.0 - CADJ))
LW_CLAMP = -50.0             # ln-weight clamp (e^{x-50} ~ 0)

F32 = mybir.dt.float32
BF16 = mybir.dt.bfloat16
I16 = mybir.dt.int16
Op = mybir.AluOpType


def _build() -> bass.Bass:
    nc = bacc.Bacc("TRN2", target_bir_lowering=False, debug=False)
    x = nc.declare_dram_parameter("x", [128, NQ * ROWS], BF16, isOutput=False)
    wv = nc.declare_dram_parameter("wv", [128, 2 * NQ], BF16, isOutput=False)
    scs = nc.declare_dram_parameter("scs", [2, ROWS], F32, isOutput=False)
    out = nc.declare_dram_parameter("out", [2, 1], F32, isOutput=True)

    with tile.TileContext(nc) as tc, ExitStack() as ctx:
        const_pool = ctx.enter_context(tc.tile_pool(name="const", bufs=1))
        xpool = ctx.enter_context(tc.tile_pool(name="x", bufs=1))
        epool = ctx.enter_context(tc.tile_pool(name="e", bufs=1))
        spool = ctx.enter_context(tc.tile_pool(name="s", bufs=1))
        psum_pool = ctx.enter_context(tc.tile_pool(name="ps", bufs=1, space="PSUM"))

        # constants ride the scalar HWDGE ring (lands ahead of the x stream)
        wvt = const_pool.tile([128, 2 * NQ], BF16)
        nc.scalar.dma_start(out=wvt[:], in_=wv[:, :])
        scst = const_pool.tile([2, ROWS], F32)
        nc.scalar.dma_start(out=scst[:], in_=scs[:, :])

        # x tiles stream on the sync ring in consumption order
        xt = []
        for j in range(NT):
            t = xpool.tile([128, W], BF16, name=f"x{j}")
            nc.sync.dma_start(out=t[:], in_=x[:, j * W:(j + 1) * W])
            xt.append(t)

        asps = psum_pool.tile([2, ROWS], F32, tag="pas")
        res = spool.tile([2, 1], F32)

        for j in range(NT):
            et = epool.tile([128, W], I16, name=f"e{j}")
            nc.vector.tensor_scalar(
                et[:], xt[j][:], SCHR_SCALE, SCHR_BIAS, op0=Op.mult, op1=Op.add
            )
            eb = et[:].bitcast(BF16)
            for k in range(SLOTS):
                q = j * SLOTS + k
                nc.tensor.matmul(
                    asps[:], wvt[:, 2 * q:2 * q + 2],
                    eb[:, k * ROWS:(k + 1) * ROWS],
                    start=(q == 0), stop=(q == NQ - 1),
                )

        j0 = spool.tile([2, ROWS], F32)
        nc.vector.scalar_tensor_tensor(
            j0[:], asps[:], 0.0, scst[:], op0=Op.add, op1=Op.mult,
            accum_out=res[:, 0:1],
        )
        nc.sync.dma_start(out=out[:, :], in_=res[:])

    nc.finalize()
    return nc


_NC = None


def _get_nc() -> bass.Bass:
    global _NC
    if _NC is None:
        _NC = _build()
    return _NC


def _plan(target: np.ndarray):
    """Sort rows by target; per core pick a contiguous 16-block window
    covering all its targets and a block permutation [agg | window]."""
    t = np.asarray(target).astype(np.int64).reshape(B)
    order = np.argsort(t, kind="stable")
    plans = []
    for k in range(N_CORES):
        rows = order[k * ROWS:(k + 1) * ROWS]
        tc = t[rows]
        blo, bhi = int(tc.min()) >> 7, int(tc.max()) >> 7
        span = bhi - blo + 1
        assert span <= NS, f"target spread too wide for window: {span} blocks"
        wlo = min(max(blo - (NS - span) // 2, 0), NQ - NS)
        assert wlo <= blo and bhi < wlo + NS
        win = np.arange(wlo, wlo + NS)
        rest = np.array([q for q in range(NQ) if q < wlo or q >= wlo + NS])
        plans.append((rows, tc, win, rest))
    return plans


def make_in_maps(input: np.ndarray, target: np.ndarray) -> list[dict]:
    xf = np.asarray(input, dtype=np.float32)
    plans = _plan(target)
    # row norm (exact closed form): sum_c (c-t)^2 = C*t^2 - 2*t*S1 + S2
    s1 = (C - 1) * C // 2
    s2 = (C - 1) * C * (2 * C - 1) // 6
    in_maps = []
    p128 = np.arange(128, dtype=np.float64)
    for rows, tc, win, rest in plans:
        # slot order: 48 agg blocks then 16 window blocks
        perm = np.concatenate([rest, win])
        m = float(win[0] * 128 + (NS * 128) / 2.0)
        # payload: agg slots = x; window slots = x + ln|c - t| (clamped)
        xk = xf[rows].reshape(ROWS, NQ, 128)[:, perm, :]  # [ROWS, slot, 128]
        xT = np.ascontiguousarray(xk.transpose(2, 1, 0))  # [128, slot, ROWS]
        cw = (win[None, :] * 128 + p128[:, None])          # [128, NS] c values
        dist = np.abs(cw[:, :, None] - tc[None, None, :].astype(np.float64))
        lw = np.log(np.maximum(dist, 1e-30))
        np.maximum(lw, LW_CLAMP, out=lw)
        xT[:, NAGG:, :] += lw.astype(np.float32)
        xb = xT.reshape(128, NQ * ROWS).astype(ml_dtypes.bfloat16)

        # stationary columns: agg [-+(c-m), -+1]; window [1, 0]
        sgn = np.where(rest * 128 > win[-1] * 128, 1.0, -1.0)
        cagg = rest[None, :] * 128 + p128[:, None] - m      # [128, NAGG]
        wvc = np.zeros((128, 2 * NQ), dtype=np.float32)
        wvc[:, 0:2 * NAGG:2] = cagg * sgn[None, :]
        wvc[:, 1:2 * NAGG:2] = sgn[None, :]
        wvc[:, 2 * NAGG::2] = 1.0

        norm = np.sqrt(C * tc.astype(np.float64) ** 2 - 2.0 * tc * s1 + s2)
        sc64 = COEFF / np.maximum(norm, 1e-12) * (tc != 0)
        scs = np.stack([sc64, sc64 * (m - tc.astype(np.float64))])
        in_maps.append({
            "x": xb,
            "wv": wvc.astype(ml_dtypes.bfloat16),
            "scs": np.ascontiguousarray(scs.astype(np.float32)),
        })
    return in_maps


def run(input: np.ndarray, target: np.ndarray, trace: bool = False, tmpdir=None):
    nc = _get_nc()
    in_maps = make_in_maps(input, target)
    res = run_bass_kernel_spmd(
        nc, in_maps, list(range(N_CORES)), trace=trace, tmpdir=tmpdir
    )
    total = np.float32(0.0)
    for r in res.results:
        total += np.float32(r["out"].reshape(-1).sum())
    return np.asarray(total, dtype=np.float32), res


def kernel(input: np.ndarray, target: np.ndarray) -> np.ndarray:
    out, _ = run(input, target)
    return out
